# revision 17
# baseline (speedup 1.0000x reference)
"""CapsNet routing layer (nn_CapsLayer) on 8 Trainium2 NeuronCores.

reference:
    u_hat = einsum("ncoi,bci->bnco", W[0], x)         # B,N,C,O = 1024,2,512,64
    3 dynamic-routing iterations (softmax over n, weighted sum over c,
    squash, agreement update); returns v from iteration 3.

Strategy (in-caps sharded, hardcoded shapes):
  - 8 cores x 64 in-caps, every core sees the full batch. GEMM is a
    2-pass bf16 decomposition (xh*W + xl*W, fp32 PSUM accumulate); u is
    stored bf16 (b=128p, c=64, no=128) per 128-sample chunk.
  - ALL routing on DVE + ACT. GPSIMD only triggers collectives: measured
    on HW it runs elementwise at ~2.7 ns/elem AND contends with DVE for
    SBUF, stretching DVE slices ~2.5x (net-negative, twice confirmed).
  - softmax over N=2 -> per-n sigmoid pair (sig(d), sig(-d)) multiplied
    directly (an ubar-minus identity cancels catastrophically when sig
    saturates); sigma kept fp32 (free: the sigma-mult broadcasts over the
    innermost axis so it runs 1x regardless).
  - Reductions are butterfly tree-adds: stage 1 in bf16 (packed 2x),
    later stages fp32 (1x, small) to keep partial-sum noise out of the
    threshold-sensitive d/sigmoid path. Agreement y0-y1 in one pass via
    w = [v0, -v1] bf16 (y-mult stays packed-2x); it2 skips the d update.
  - Cross-core s-reduction: one fp32 AllReduce per (3-chunk group, iter)
    = 9 total (CC stream ~6 GB/s, ~10us each), pipelined group-serially
    at depth 2 so CC latency hides behind the other group's routing.
"""
import os
import sys
import types

sys.path.insert(0, "/opt/trn_rl_repo")

import numpy as np
import ml_dtypes
import concourse.bass as bass
import concourse.mybir as mybir
import concourse.tile as tile
from concourse.bass_utils import run_bass_kernel_spmd

BF16NP = ml_dtypes.bfloat16

B, NCAPS, C, ICH, OCH = 1024, 2, 512, 256, 64
NO = NCAPS * OCH             # 128 fused (n, o)
ITERATIONS = 3
NCORES = 8
CPC = C // NCORES            # in-caps per core = 64
NBCH = 8                     # batch chunks
BCH = B // NBCH              # samples per chunk = 128
KH = 2                       # K halves (ICH = 2*128)
CG = 8                       # c's per GEMM/PSUM group

FP32 = mybir.dt.float32
BF16 = mybir.dt.bfloat16
ADD = mybir.AluOpType.add
MULT = mybir.AluOpType.mult
SUB = mybir.AluOpType.subtract
AF = mybir.ActivationFunctionType
AX = mybir.AxisListType

GROUPS = [[0, 1], [2, 3], [4, 5], [6, 7]]

LAST_EXEC_NS = None


def _install_profile_hook():
    """antenv.axon_hooks is absent in this image; recreate it so
    run_bass_kernel_spmd(trace=True)/BASS_TRACE can report exec_time_ns."""
    if "antenv.axon_hooks" in sys.modules:
        return
    mod = types.ModuleType("antenv.axon_hooks")
    mod._hook = None
    mod.set_axon_ntff_profile_hook = lambda h: setattr(mod, "_hook", h)
    mod.get_axon_ntff_profile_hook = lambda: mod._hook
    sys.modules["antenv.axon_hooks"] = mod
    try:
        from trn_agent_boot.trn_boot import _ntff_profile_via_ctypes

        hook = _ntff_profile_via_ctypes("/opt/axon/libaxon_pjrt.so")
        if hook is not None:
            mod._hook = hook
    except Exception:
        pass


def _split_sync_waits(nc, max_waits=1):
    """walrus setupSyncWait rejects instructions with more than one sem
    wait; hoist extras onto same-engine InstNoOp's placed just before."""
    for f in nc.m.functions:
        for bb in f.blocks:
            out = []
            changed = False
            for inst in bb.instructions:
                si = inst.sync_info
                waits = list(si.on_wait) if si is not None and si.on_wait else []
                if len(waits) > max_waits:
                    extra, keep = waits[:-max_waits], waits[-max_waits:]
                    for g, w in enumerate(extra):
                        out.append(
                            mybir.InstNoOp(
                                name=f"{inst.name}_wsplit{g}",
                                engine=inst.engine,
                                bass_nofuse=True,
                                sync_info=mybir.SyncInfo(on_wait=[w], on_update=[]),
                            )
                        )
                    inst.sync_info = mybir.SyncInfo(
                        on_wait=keep,
                        on_update=list(si.on_update) if si.on_update else [],
                    )
                    changed = True
                out.append(inst)
            if changed:
                bb.instructions = out


def build_kernel(split_waits=True):
    nc = bass.Bass(
        "TRN2", target_bir_lowering=False, debug=False, num_devices=NCORES
    )
    # x shard hi/lo: [h, i, bchunk, c, b] bf16
    xth = nc.dram_tensor("xth", [KH, 128, NBCH, CPC, BCH], BF16, kind="ExternalInput").ap()
    xtl = nc.dram_tensor("xtl", [KH, 128, NBCH, CPC, BCH], BF16, kind="ExternalInput").ap()
    # W shard: [h, i, c, (n,o)] bf16
    wth = nc.dram_tensor("wth", [KH, 128, CPC, NO], BF16, kind="ExternalInput").ap()
    out = nc.dram_tensor("out", [B, NCAPS, OCH], FP32, kind="ExternalOutput").ap()

    with tile.TileContext(nc) as tc:
        with (
            tc.tile_pool(name="xin", bufs=2) as xpool,
            tc.tile_pool(name="psum", bufs=2, space="PSUM") as pspool,
            tc.tile_pool(name="ubuf", bufs=6) as upool,
            tc.tile_pool(name="tbuf", bufs=1) as tpool,      # sigma*u / u*w product (DVE only)
            tc.tile_pool(name="bfly", bufs=1) as fpool,      # tree stages (DVE only)
            tc.tile_pool(name="smal", bufs=2) as spool,      # fp32 smalls
            tc.tile_pool(name="dram", bufs=4, space="DRAM") as drpool,
        ):
            # resident W: per K-half tiles (128i, c*no)
            wsb = {}
            for h in range(KH):
                t = nc.alloc_sbuf_tensor(f"w{h}", [128, CPC * NO], BF16).ap()
                nc.sync.dma_start(t[:], wth[h].rearrange("i c f -> i (c f)"))
                wsb[h] = t

            # per-chunk persistent routing state
            d_all = nc.alloc_sbuf_tensor("d_all", [128, NBCH, CPC], FP32).ap()
            sig_all = nc.alloc_sbuf_tensor("sig_all", [128, NBCH, CPC, NCAPS], BF16).ap()
            w_all = nc.alloc_sbuf_tensor("w_all", [128, NBCH, NO], BF16).ap()
            # s partial before AllReduce; the reduced result lands in the
            # same per-chunk slot (the CC input DMA completes first)
            srp_all = nc.alloc_sbuf_tensor("srp_all", [128, NBCH, NO], FP32).ap()
            # tiny squash scalars per chunk
            q_all = nc.alloc_sbuf_tensor("q_all", [128, NBCH, NCAPS], FP32).ap()
            rr_all = nc.alloc_sbuf_tensor("rr_all", [128, NBCH, NCAPS], FP32).ap()
            den_all = nc.alloc_sbuf_tensor("den_all", [128, NBCH, NCAPS], FP32).ap()
            rec_all = nc.alloc_sbuf_tensor("rec_all", [128, NBCH, NCAPS], FP32).ap()
            mv_all = nc.alloc_sbuf_tensor("mv_all", [128, NBCH, NCAPS], FP32).ap()

            us = {}

            def gemm_chunk(bk):
                u = upool.tile([128, CPC, NO], BF16, tag="u")
                for cg in range(CPC // CG):
                    c0 = cg * CG
                    xt = {}
                    for h in range(KH):
                        for hl, src in (("h", xth), ("l", xtl)):
                            t = xpool.tile([128, CG, BCH], BF16, tag=f"x{hl}{h}")
                            nc.sync.dma_start(t[:], src[h, :, bk, c0 : c0 + CG, :])
                            xt[(h, hl)] = t
                    pg = pspool.tile([BCH, CG, NO], FP32, tag="pg")
                    for j in range(CG):
                        c = c0 + j
                        terms = []
                        for h in range(KH):
                            terms.append((xt[(h, "h")][:, j, :], wsb[h]))
                            terms.append((xt[(h, "l")][:, j, :], wsb[h]))
                        for ti, (sta, wt_) in enumerate(terms):
                            nc.tensor.matmul(
                                pg[:, j, :],
                                lhsT=sta,
                                rhs=wt_[:, c * NO : (c + 1) * NO],
                                start=(ti == 0),
                                stop=(ti == len(terms) - 1),
                            )
                    nc.scalar.copy(u[:, c0 : c0 + CG, :], pg[:])
                us[bk] = u

            def _stage(nelem, dt, a, b_, view):
                """tree stage: flat scratch tile viewed as `view`, adds a+b_."""
                tag = f"s{'b' if dt == BF16 else 'f'}{nelem}"
                flat = fpool.tile([128, nelem], dt, tag=tag)
                o = flat[:].rearrange(f"p (a b) -> p a b", a=view[0])
                nc.vector.tensor_tensor(o, a, b_, op=ADD)
                return o

            def tree_c(src, dst_fp32):
                """sum (128, 64, NO) over dim1 -> dst (128, NO) fp32.
                All stages fp32 (bf16 partial sums here cost ~0.6e-2 absmax);
                run as two c-half trees + join so scratch stays at 8 KB."""
                phs = []
                for hf in range(2):
                    cs = hf * 32
                    cur = _stage(
                        16 * NO, FP32,
                        src[:, cs : cs + 16, :], src[:, cs + 16 : cs + 32, :],
                        (16, NO),
                    )
                    n = 16
                    while n > 2:
                        h = n // 2
                        cur = _stage(
                            h * NO, FP32, cur[:, 0:h, :], cur[:, h : 2 * h, :], (h, NO)
                        )
                        n = h
                    ph = fpool.tile([128, NO], FP32, tag=f"ph{hf}")
                    nc.vector.tensor_tensor(
                        ph[:].unsqueeze(1), cur[:, 0:1, :], cur[:, 1:2, :], op=ADD
                    )
                    phs.append(ph)
                nc.vector.tensor_tensor(
                    dst_fp32, phs[0][:], phs[1][:], op=ADD
                )

            def tree_no(src, dst_fp32):
                """sum (128, CPC, 128) over dim2 -> dst (128, CPC) fp32."""
                cur = _stage(
                    CPC * 64, BF16, src[:, :, 0:64], src[:, :, 64:128], (CPC, 64)
                )
                n = 64
                while n > 2:
                    h = n // 2
                    cur = _stage(
                        CPC * h, FP32, cur[:, :, 0:h], cur[:, :, h : 2 * h], (CPC, h)
                    )
                    n = h
                nc.vector.tensor_tensor(
                    dst_fp32.unsqueeze(2), cur[:, :, 0:1], cur[:, :, 1:2], op=ADD
                )

            def phase_A(bk, it):
                """produce s'-partial (sum over this core's c) into srp_all."""
                u = us[bk]
                if it == 0:
                    tree_c(u, srp_all[:, bk, :])
                else:
                    # sigma expanded to bf16 on ACT per c-half so the DVE
                    # mult is all-bf16 packed (2x) instead of a 1x broadcast
                    t = tpool.tile([128, CPC, NO], BF16, tag="t")
                    for hf in range(2):
                        cs = hf * 32
                        sx = fpool.tile([128, 32 * NO], BF16, tag="sx")
                        sxv = sx[:].rearrange("p (c n o) -> p c n o", c=32, n=NCAPS)
                        nc.scalar.copy(
                            sxv,
                            sig_all[:, bk, cs : cs + 32].unsqueeze(3).broadcast_to(
                                (128, 32, NCAPS, OCH)
                            ),
                        )
                        nc.vector.tensor_tensor(
                            t[:, cs : cs + 32, :],
                            u[:, cs : cs + 32, :],
                            sx[:].rearrange("p (c f) -> p c f", c=32),
                            op=MULT,
                        )
                    tree_c(t[:], srp_all[:, bk, :])

            def collective(g, it):
                ng = len(g)
                bi = drpool.tile([128, ng * NO], FP32, tag="bi")
                bo = drpool.tile([128, ng * NO], FP32, tag="bo")
                for i, bk in enumerate(g):
                    nc.sync.dma_start(bi[:, i * NO : (i + 1) * NO], srp_all[:, bk, :])
                nc.gpsimd.collective_compute(
                    "AllReduce",
                    ADD,
                    replica_groups=[list(range(NCORES))],
                    ins=[bi[:].opt()],
                    outs=[bo[:].opt()],
                )
                for i, bk in enumerate(g):
                    nc.sync.dma_start(srp_all[:, bk, :], bo[:, i * NO : (i + 1) * NO])

            def phase_B(bk, it):
                """post-AllReduce: squash -> v; then w, y-pass, d, sigmas."""
                u = us[bk]
                sr = srp_all[:, bk, :]
                if it == 0:
                    sful = spool.tile([128, NO], FP32, tag="sful")
                    nc.vector.tensor_scalar(sful[:], sr, 0.5, None, op0=MULT)
                    sful = sful[:]
                else:
                    # read sr in place: this chunk's srp slot is not rewritten
                    # until its own A(it+1), emitted after this phase
                    sful = sr
                # squash: v = s * q / ((1+q) sqrt(q)), q = |s|^2 per (b, n)
                s2 = spool.tile([128, NO], FP32, tag="s2")
                q = q_all[:, bk, :]
                rr = rr_all[:, bk, :]
                den = den_all[:, bk, :]
                rec = rec_all[:, bk, :]
                mv = mv_all[:, bk, :]
                nc.vector.tensor_tensor(s2[:], sful[:], sful[:], op=MULT)
                nc.vector.tensor_reduce(
                    q.unsqueeze(2),
                    s2[:].rearrange("b (n o) -> b n o", n=NCAPS),
                    axis=AX.X,
                    op=ADD,
                )
                nc.scalar.activation(rr, q, AF.Sqrt)
                nc.vector.tensor_scalar(den, q, 1.0, None, op0=ADD)
                nc.vector.tensor_tensor(den, den, rr, op=MULT)
                nc.vector.reciprocal(rec, den)
                nc.vector.tensor_tensor(mv, q, rec, op=MULT)
                v = spool.tile([128, NCAPS, OCH], FP32, tag="v")
                nc.vector.tensor_tensor(
                    v[:],
                    sful[:].rearrange("b (n o) -> b n o", n=NCAPS),
                    mv.unsqueeze(2).broadcast_to((128, NCAPS, OCH)),
                    op=MULT,
                )
                if it == ITERATIONS - 1:
                    nc.sync.dma_start(out[bk * BCH : (bk + 1) * BCH, :, :], v[:])
                    return
                # w = [v0, -v1] bf16
                w = w_all[:, bk, :]
                nc.scalar.copy(w[:, 0:OCH], v[:, 0, :])
                nc.vector.tensor_scalar(w[:, OCH:NO], v[:, 1, :], -1.0, None, op0=MULT)
                # y pass: t = u * w (packed 2x: w broadcasts over middle c)
                t = tpool.tile([128, CPC, NO], BF16, tag="t")
                nc.vector.tensor_tensor(
                    t[:], u[:], w.unsqueeze(1).broadcast_to((128, CPC, NO)), op=MULT
                )
                d = d_all[:, bk, :]
                if it == 0:
                    # single fp32-accumulating reduce over innermost no (the
                    # it0 agreement seeds d; oracle wants it fp32-clean)
                    nc.vector.tensor_reduce(
                        d.unsqueeze(2), t[:], axis=AX.X, op=ADD
                    )
                else:
                    dd = spool.tile([128, CPC], FP32, tag="dd")
                    tree_no(t[:], dd[:])
                    nc.vector.tensor_tensor(d, d, dd[:], op=ADD)
                nc.scalar.activation(sig_all[:, bk, :, 0], d, AF.Sigmoid)
                nc.scalar.activation(sig_all[:, bk, :, 1], d, AF.Sigmoid, scale=-1.0)

            def A_CC(gi, it):
                if it == 0:
                    # per-chunk collectives: it0 partials return during the
                    # GEMM fill so squash/y0 work feeds the otherwise-idle DVE
                    for k in GROUPS[gi]:
                        phase_A(k, it)
                        collective([k], it)
                else:
                    for k in GROUPS[gi]:
                        phase_A(k, it)
                    collective(GROUPS[gi], it)

            def B_(gi, it):
                for k in GROUPS[gi]:
                    phase_B(k, it)

            # group-serial, depth-2 software pipeline (u-pool bufs=6 gates
            # group 2's gemm behind A(g0, it2), which is emitted earlier)
            for k in GROUPS[0]:
                gemm_chunk(k)
            A_CC(0, 0)
            for k in GROUPS[1]:
                gemm_chunk(k)
            A_CC(1, 0)
            B_(0, 0); A_CC(0, 1)
            B_(1, 0); A_CC(1, 1)
            B_(0, 1); A_CC(0, 2)
            for k in GROUPS[2]:
                gemm_chunk(k)
            A_CC(2, 0)
            B_(1, 1); A_CC(1, 2)
            for k in GROUPS[3]:
                gemm_chunk(k)
            A_CC(3, 0)
            B_(0, 2)
            B_(2, 0); A_CC(2, 1)
            B_(1, 2)
            B_(2, 1); A_CC(2, 2)
            B_(3, 0); A_CC(3, 1)
            B_(2, 2)
            B_(3, 1); A_CC(3, 2)
            B_(3, 2)

    if split_waits:
        _split_sync_waits(nc)
    return nc


def _prep_inputs(x, W):
    x = np.ascontiguousarray(x, dtype=np.float32)
    W0 = np.ascontiguousarray(W.reshape(NCAPS, C, OCH, ICH), dtype=np.float32)
    xth_cores, xtl_cores, wth_cores = [], [], []
    for k in range(NCORES):
        cs = k * CPC
        xc = x[:, cs : cs + CPC, :]  # (B, 64, 256)
        x6 = xc.reshape(NBCH, BCH, CPC, KH, 128)
        xt = np.ascontiguousarray(x6.transpose(3, 4, 0, 2, 1))  # (h,i,bk,c,b)
        xh = xt.astype(BF16NP)
        xlo = (xt - xh.astype(np.float32)).astype(BF16NP)
        xth_cores.append(xh)
        xtl_cores.append(xlo)
        Wc = W0[:, cs : cs + CPC]  # (2, 64, 64, 256)
        w5 = Wc.reshape(NCAPS, CPC, OCH, KH, 128)
        wt = np.ascontiguousarray(w5.transpose(3, 4, 1, 0, 2)).reshape(
            KH, 128, CPC, NO
        )
        wth_cores.append(wt.astype(BF16NP))
    return xth_cores, xtl_cores, wth_cores


_NC_CACHE = {}


def kernel(x, W):
    global LAST_EXEC_NS
    _install_profile_hook()
    if "nc" not in _NC_CACHE:
        _NC_CACHE["nc"] = build_kernel()
    nc = _NC_CACHE["nc"]
    xth, xtl, wth = _prep_inputs(np.asarray(x), np.asarray(W))
    in_maps = [
        {"xth": xth[k], "xtl": xtl[k], "wth": wth[k]} for k in range(NCORES)
    ]
    trace = bool(os.environ.get("CAPS_TRACE"))
    res = run_bass_kernel_spmd(nc, in_maps, list(range(NCORES)), trace=trace)
    LAST_EXEC_NS = res.exec_time_ns
    return res.results[0]["out"].astype(np.float32)


# revision 19
# speedup vs baseline: 1.2209x; 1.2209x over previous
"""CapsNet routing layer (nn_CapsLayer) on 8 Trainium2 NeuronCores.

reference:
    u_hat = einsum("ncoi,bci->bnco", W[0], x)         # B,N,C,O = 1024,2,512,64
    3 dynamic-routing iterations (softmax over n, weighted sum over c,
    squash, agreement update); returns v from iteration 3.

Strategy (in-caps sharded, hardcoded shapes):
  - 8 cores x 64 in-caps, every core sees the full batch. GEMM is a
    2-pass bf16 decomposition (xh*W + xl*W, fp32 PSUM accumulate); u is
    stored bf16 (b=128p, c=64, no=128) per 128-sample chunk.
  - ALL routing on DVE + ACT. GPSIMD only triggers collectives: measured
    on HW it runs elementwise at ~2.7 ns/elem AND contends with DVE for
    SBUF, stretching DVE slices ~2.5x (net-negative, twice confirmed).
  - softmax over N=2 -> per-n sigmoid pair (sig(d), sig(-d)) multiplied
    directly (an ubar-minus identity cancels catastrophically when sig
    saturates); sigma kept fp32 (free: the sigma-mult broadcasts over the
    innermost axis so it runs 1x regardless).
  - Reductions are butterfly tree-adds: stage 1 in bf16 (packed 2x),
    later stages fp32 (1x, small) to keep partial-sum noise out of the
    threshold-sensitive d/sigmoid path. Agreement y0-y1 in one pass via
    w = [v0, -v1] bf16 (y-mult stays packed-2x); it2 skips the d update.
  - Cross-core s-reduction: one fp32 AllReduce per (3-chunk group, iter)
    = 9 total (CC stream ~6 GB/s, ~10us each), pipelined group-serially
    at depth 2 so CC latency hides behind the other group's routing.
"""
import os
import sys
import types

sys.path.insert(0, "/opt/trn_rl_repo")

import numpy as np
import ml_dtypes
import concourse.bass as bass
import concourse.mybir as mybir
import concourse.tile as tile
from concourse.bass_utils import run_bass_kernel_spmd

F16NP = np.float16

B, NCAPS, C, ICH, OCH = 1024, 2, 512, 256, 64
NO = NCAPS * OCH             # 128 fused (n, o)
ITERATIONS = 3
NCORES = 8
CPC = C // NCORES            # in-caps per core = 64
NBCH = 8                     # batch chunks
BCH = B // NBCH              # samples per chunk = 128
KH = 2                       # K halves (ICH = 2*128)
CG = 8                       # c's per GEMM/PSUM group

FP32 = mybir.dt.float32
BF16 = mybir.dt.bfloat16
FP16 = mybir.dt.float16
ADD = mybir.AluOpType.add
MULT = mybir.AluOpType.mult
SUB = mybir.AluOpType.subtract
AF = mybir.ActivationFunctionType
AX = mybir.AxisListType

GROUPS = [[0, 1], [2, 3], [4, 5], [6, 7]]

LAST_EXEC_NS = None


def _install_profile_hook():
    """antenv.axon_hooks is absent in this image; recreate it so
    run_bass_kernel_spmd(trace=True)/BASS_TRACE can report exec_time_ns."""
    if "antenv.axon_hooks" in sys.modules:
        return
    mod = types.ModuleType("antenv.axon_hooks")
    mod._hook = None
    mod.set_axon_ntff_profile_hook = lambda h: setattr(mod, "_hook", h)
    mod.get_axon_ntff_profile_hook = lambda: mod._hook
    sys.modules["antenv.axon_hooks"] = mod
    try:
        from trn_agent_boot.trn_boot import _ntff_profile_via_ctypes

        hook = _ntff_profile_via_ctypes("/opt/axon/libaxon_pjrt.so")
        if hook is not None:
            mod._hook = hook
    except Exception:
        pass


def _split_sync_waits(nc, max_waits=1):
    """walrus setupSyncWait rejects instructions with more than one sem
    wait; hoist extras onto same-engine InstNoOp's placed just before."""
    for f in nc.m.functions:
        for bb in f.blocks:
            out = []
            changed = False
            for inst in bb.instructions:
                si = inst.sync_info
                waits = list(si.on_wait) if si is not None and si.on_wait else []
                if len(waits) > max_waits:
                    extra, keep = waits[:-max_waits], waits[-max_waits:]
                    for g, w in enumerate(extra):
                        out.append(
                            mybir.InstNoOp(
                                name=f"{inst.name}_wsplit{g}",
                                engine=inst.engine,
                                bass_nofuse=True,
                                sync_info=mybir.SyncInfo(on_wait=[w], on_update=[]),
                            )
                        )
                    inst.sync_info = mybir.SyncInfo(
                        on_wait=keep,
                        on_update=list(si.on_update) if si.on_update else [],
                    )
                    changed = True
                out.append(inst)
            if changed:
                bb.instructions = out


def build_kernel(split_waits=True):
    nc = bass.Bass(
        "TRN2", target_bir_lowering=False, debug=False, num_devices=NCORES
    )
    # x shard hi/lo: [h, i, bchunk, c, b] bf16
    xth = nc.dram_tensor("xth", [KH, 128, NBCH, CPC, BCH], FP16, kind="ExternalInput").ap()
    # W shard: [h, i, c, (n,o)] fp16
    wth = nc.dram_tensor("wth", [KH, 128, CPC, NO], FP16, kind="ExternalInput").ap()
    out = nc.dram_tensor("out", [B, NCAPS, OCH], FP32, kind="ExternalOutput").ap()

    with tile.TileContext(nc) as tc:
        with (
            tc.tile_pool(name="xin", bufs=2) as xpool,
            tc.tile_pool(name="psum", bufs=2, space="PSUM") as pspool,
            tc.tile_pool(name="ubuf", bufs=6) as upool,
            tc.tile_pool(name="tbuf", bufs=1) as tpool,      # sigma*u / u*w product (DVE only)
            tc.tile_pool(name="bfly", bufs=1) as fpool,      # tree stages (DVE only)
            tc.tile_pool(name="smal", bufs=2) as spool,      # fp32 smalls
            tc.tile_pool(name="dram", bufs=4, space="DRAM") as drpool,
        ):
            # resident W: per K-half tiles (128i, c*no)
            wsb = {}
            for h in range(KH):
                t = nc.alloc_sbuf_tensor(f"w{h}", [128, CPC * NO], FP16).ap()
                nc.sync.dma_start(t[:], wth[h].rearrange("i c f -> i (c f)"))
                wsb[h] = t

            # per-chunk persistent routing state
            d_all = nc.alloc_sbuf_tensor("d_all", [128, NBCH, CPC], FP32).ap()
            sig_all = nc.alloc_sbuf_tensor("sig_all", [128, NBCH, CPC, NCAPS], FP16).ap()
            w_all = nc.alloc_sbuf_tensor("w_all", [128, NBCH, NO], FP16).ap()
            # s partial before AllReduce; the reduced result lands in the
            # same per-chunk slot (the CC input DMA completes first)
            srp_all = nc.alloc_sbuf_tensor("srp_all", [128, NBCH, NO], FP32).ap()
            # tiny squash scalars per chunk
            q_all = nc.alloc_sbuf_tensor("q_all", [128, NBCH, NCAPS], FP32).ap()
            rr_all = nc.alloc_sbuf_tensor("rr_all", [128, NBCH, NCAPS], FP32).ap()
            den_all = nc.alloc_sbuf_tensor("den_all", [128, NBCH, NCAPS], FP32).ap()
            rec_all = nc.alloc_sbuf_tensor("rec_all", [128, NBCH, NCAPS], FP32).ap()
            mv_all = nc.alloc_sbuf_tensor("mv_all", [128, NBCH, NCAPS], FP32).ap()

            us = {}

            def gemm_chunk(bk):
                u = upool.tile([128, CPC, NO], FP16, tag="u")
                for cg in range(CPC // CG):
                    c0 = cg * CG
                    xt = {}
                    for h in range(KH):
                        t = xpool.tile([128, CG, BCH], FP16, tag=f"x{h}")
                        nc.sync.dma_start(t[:], xth[h, :, bk, c0 : c0 + CG, :])
                        xt[h] = t
                    pg = pspool.tile([BCH, CG, NO], FP32, tag="pg")
                    for j in range(CG):
                        c = c0 + j
                        for h in range(KH):
                            nc.tensor.matmul(
                                pg[:, j, :],
                                lhsT=xt[h][:, j, :],
                                rhs=wsb[h][:, c * NO : (c + 1) * NO],
                                start=(h == 0),
                                stop=(h == KH - 1),
                            )
                    nc.scalar.copy(u[:, c0 : c0 + CG, :], pg[:])
                us[bk] = u

            def _stage(nelem, dt, a, b_, view):
                """tree stage: flat scratch tile viewed as `view`, adds a+b_."""
                tag = f"s{'h' if dt == FP16 else 'f'}{nelem}"
                flat = fpool.tile([128, nelem], dt, tag=tag)
                o = flat[:].rearrange(f"p (a b) -> p a b", a=view[0])
                nc.vector.tensor_tensor(o, a, b_, op=ADD)
                return o

            def tree_c(src, dst_fp32):
                """sum (128, 64, NO) over dim1 -> dst (128, NO) fp32.
                fp16 stages (packed 2x; values are range-bounded so fp16's
                10-bit mantissa keeps partial-sum noise ~8x below bf16),
                final join fp32."""
                cur = _stage(32 * NO, FP16, src[:, 0:32, :], src[:, 32:64, :], (32, NO))
                n = 32
                while n > 2:
                    h = n // 2
                    cur = _stage(
                        h * NO, FP16, cur[:, 0:h, :], cur[:, h : 2 * h, :], (h, NO)
                    )
                    n = h
                nc.vector.tensor_tensor(
                    dst_fp32.unsqueeze(1), cur[:, 0:1, :], cur[:, 1:2, :], op=ADD
                )

            def tree_no(src, dst_fp32):
                """sum (128, CPC, 128) over dim2 -> dst (128, CPC) fp32."""
                cur = _stage(
                    CPC * 64, FP16, src[:, :, 0:64], src[:, :, 64:128], (CPC, 64)
                )
                n = 64
                while n > 2:
                    h = n // 2
                    cur = _stage(
                        CPC * h, FP16, cur[:, :, 0:h], cur[:, :, h : 2 * h], (CPC, h)
                    )
                    n = h
                nc.vector.tensor_tensor(
                    dst_fp32.unsqueeze(2), cur[:, :, 0:1], cur[:, :, 1:2], op=ADD
                )

            def phase_A(bk, it):
                """produce s'-partial (sum over this core's c) into srp_all."""
                u = us[bk]
                if it == 0:
                    tree_c(u, srp_all[:, bk, :])
                else:
                    # sigma expanded to bf16 on ACT per c-half so the DVE
                    # mult is all-bf16 packed (2x) instead of a 1x broadcast
                    t = tpool.tile([128, CPC, NO], FP16, tag="t")
                    for hf in range(2):
                        cs = hf * 32
                        sx = fpool.tile([128, 32 * NO], FP16, tag="sx")
                        sxv = sx[:].rearrange("p (c n o) -> p c n o", c=32, n=NCAPS)
                        nc.scalar.copy(
                            sxv,
                            sig_all[:, bk, cs : cs + 32].unsqueeze(3).broadcast_to(
                                (128, 32, NCAPS, OCH)
                            ),
                        )
                        nc.vector.tensor_tensor(
                            t[:, cs : cs + 32, :],
                            u[:, cs : cs + 32, :],
                            sx[:].rearrange("p (c f) -> p c f", c=32),
                            op=MULT,
                        )
                    tree_c(t[:], srp_all[:, bk, :])

            def collective(g, it):
                ng = len(g)
                bi = drpool.tile([128, ng * NO], FP32, tag="bi")
                bo = drpool.tile([128, ng * NO], FP32, tag="bo")
                for i, bk in enumerate(g):
                    nc.sync.dma_start(bi[:, i * NO : (i + 1) * NO], srp_all[:, bk, :])
                nc.gpsimd.collective_compute(
                    "AllReduce",
                    ADD,
                    replica_groups=[list(range(NCORES))],
                    ins=[bi[:].opt()],
                    outs=[bo[:].opt()],
                )
                for i, bk in enumerate(g):
                    nc.sync.dma_start(srp_all[:, bk, :], bo[:, i * NO : (i + 1) * NO])

            def phase_B(bk, it):
                """post-AllReduce: squash -> v; then w, y-pass, d, sigmas."""
                u = us[bk]
                sr = srp_all[:, bk, :]
                if it == 0:
                    sful = spool.tile([128, NO], FP32, tag="sful")
                    nc.vector.tensor_scalar(sful[:], sr, 0.5, None, op0=MULT)
                    sful = sful[:]
                else:
                    # read sr in place: this chunk's srp slot is not rewritten
                    # until its own A(it+1), emitted after this phase
                    sful = sr
                # squash: v = s * q / ((1+q) sqrt(q)), q = |s|^2 per (b, n)
                s2 = spool.tile([128, NO], FP32, tag="s2")
                q = q_all[:, bk, :]
                rr = rr_all[:, bk, :]
                den = den_all[:, bk, :]
                rec = rec_all[:, bk, :]
                mv = mv_all[:, bk, :]
                nc.vector.tensor_tensor(s2[:], sful[:], sful[:], op=MULT)
                nc.vector.tensor_reduce(
                    q.unsqueeze(2),
                    s2[:].rearrange("b (n o) -> b n o", n=NCAPS),
                    axis=AX.X,
                    op=ADD,
                )
                nc.scalar.activation(rr, q, AF.Sqrt)
                nc.vector.tensor_scalar(den, q, 1.0, None, op0=ADD)
                nc.vector.tensor_tensor(den, den, rr, op=MULT)
                nc.vector.reciprocal(rec, den)
                nc.vector.tensor_tensor(mv, q, rec, op=MULT)
                v = spool.tile([128, NCAPS, OCH], FP32, tag="v")
                nc.vector.tensor_tensor(
                    v[:],
                    sful[:].rearrange("b (n o) -> b n o", n=NCAPS),
                    mv.unsqueeze(2).broadcast_to((128, NCAPS, OCH)),
                    op=MULT,
                )
                if it == ITERATIONS - 1:
                    nc.sync.dma_start(out[bk * BCH : (bk + 1) * BCH, :, :], v[:])
                    return
                # w = [v0, -v1] bf16
                w = w_all[:, bk, :]
                nc.scalar.copy(w[:, 0:OCH], v[:, 0, :])
                nc.vector.tensor_scalar(w[:, OCH:NO], v[:, 1, :], -1.0, None, op0=MULT)
                # y pass: t = u * w (packed 2x: w broadcasts over middle c)
                t = tpool.tile([128, CPC, NO], FP16, tag="t")
                nc.vector.tensor_tensor(
                    t[:], u[:], w.unsqueeze(1).broadcast_to((128, CPC, NO)), op=MULT
                )
                d = d_all[:, bk, :]
                if it == 0:
                    tree_no(t[:], d)
                else:
                    dd = spool.tile([128, CPC], FP32, tag="dd")
                    tree_no(t[:], dd[:])
                    nc.vector.tensor_tensor(d, d, dd[:], op=ADD)
                nc.scalar.activation(sig_all[:, bk, :, 0], d, AF.Sigmoid)
                nc.scalar.activation(sig_all[:, bk, :, 1], d, AF.Sigmoid, scale=-1.0)

            def A_CC(gi, it):
                if it == 0:
                    # per-chunk collectives: it0 partials return during the
                    # GEMM fill so squash/y0 work feeds the otherwise-idle DVE
                    for k in GROUPS[gi]:
                        phase_A(k, it)
                        collective([k], it)
                else:
                    for k in GROUPS[gi]:
                        phase_A(k, it)
                    collective(GROUPS[gi], it)

            def B_(gi, it):
                for k in GROUPS[gi]:
                    phase_B(k, it)

            # group-serial, depth-2 software pipeline (u-pool bufs=6 gates
            # group 2's gemm behind A(g0, it2), which is emitted earlier)
            for k in GROUPS[0]:
                gemm_chunk(k)
            A_CC(0, 0)
            for k in GROUPS[1]:
                gemm_chunk(k)
            A_CC(1, 0)
            B_(0, 0); A_CC(0, 1)
            B_(1, 0); A_CC(1, 1)
            B_(0, 1); A_CC(0, 2)
            for k in GROUPS[2]:
                gemm_chunk(k)
            A_CC(2, 0)
            B_(1, 1); A_CC(1, 2)
            for k in GROUPS[3]:
                gemm_chunk(k)
            A_CC(3, 0)
            B_(0, 2)
            B_(2, 0); A_CC(2, 1)
            B_(1, 2)
            B_(2, 1); A_CC(2, 2)
            B_(3, 0); A_CC(3, 1)
            B_(2, 2)
            B_(3, 1); A_CC(3, 2)
            B_(3, 2)

    if split_waits:
        _split_sync_waits(nc)
    return nc


def _prep_inputs(x, W):
    x = np.ascontiguousarray(x, dtype=np.float32)
    W0 = np.ascontiguousarray(W.reshape(NCAPS, C, OCH, ICH), dtype=np.float32)
    xth_cores, wth_cores = [], []
    for k in range(NCORES):
        cs = k * CPC
        xc = x[:, cs : cs + CPC, :]  # (B, 64, 256)
        x6 = xc.reshape(NBCH, BCH, CPC, KH, 128)
        xt = np.ascontiguousarray(x6.transpose(3, 4, 0, 2, 1))  # (h,i,bk,c,b)
        xth_cores.append(xt.astype(F16NP))
        Wc = W0[:, cs : cs + CPC]  # (2, 64, 64, 256)
        w5 = Wc.reshape(NCAPS, CPC, OCH, KH, 128)
        wt = np.ascontiguousarray(w5.transpose(3, 4, 1, 0, 2)).reshape(
            KH, 128, CPC, NO
        )
        wth_cores.append(wt.astype(F16NP))
    return xth_cores, wth_cores


_NC_CACHE = {}


def kernel(x, W):
    global LAST_EXEC_NS
    _install_profile_hook()
    if "nc" not in _NC_CACHE:
        _NC_CACHE["nc"] = build_kernel()
    nc = _NC_CACHE["nc"]
    xth, wth = _prep_inputs(np.asarray(x), np.asarray(W))
    in_maps = [{"xth": xth[k], "wth": wth[k]} for k in range(NCORES)]
    trace = bool(os.environ.get("CAPS_TRACE"))
    res = run_bass_kernel_spmd(nc, in_maps, list(range(NCORES)), trace=trace)
    LAST_EXEC_NS = res.exec_time_ns
    return res.results[0]["out"].astype(np.float32)
